# revision 25
# baseline (speedup 1.0000x reference)
"""Trainium2 Bass kernel for nn_Corr (correlation-attention module).

Math (per batch n):
    f1 = 0.5*(w1 @ feat + b1)        # [4, 6400]   feat = feature_in[n] flattened
    f2 =      w2 @ feat + b2         # [4, 6400]
    S  = f1^T @ f2                   # [6400, 6400]  (0.5 = 1/sqrt(nclass) folded into f1)
    A  = softmax(S, axis=1)          # row softmax (over q)
    V  = bilinear_resize(out[n])     # [4, 6400]
    fina[c, q] = sum_p V[c, p]/Z_p * exp(S[p, q])

Sharding: 2 batches x 4 p-shards (rows of S) = 8 cores. Each core produces a
partial fina over its 1600 p-rows; host sums the 4 partials per batch.

Device kernel per core (p-shard of 1664 rows incl pad, all 6400 q):
  - S chunk = matmul(lhsT=f1[12, pblock-128cols], rhs=f2[12, qchunk]) with
    K=12 (hi/lo fp16 split rows, no zero padding needed) and M=128: one rhs
    stream per 512 cols.  PSUM: 2 ping-pong [128,1024] round buffers.
  - exp on ScalarE PSUM->SBUF (1024-wide) with accum_out giving row sums Z.
  - fina = matmul(lhsT=vt*recip(Z) [128,4], rhs=E chunks) accumulated across
    all 13 p-blocks directly in a persistent 4-bank PSUM tile (start on
    pb==0, stop on pb==12), col-stacked: partition group 32g holds q-group g.
  - fina matmuls of block b-1 are emitted inside block b's round loop
    (software pipeline) so the Scalar engine never starves.
"""

import numpy as np

N_CORES = 8
NB = 2          # batches
NCLS = 4        # nclass
C_IN = 32
H = W = 80
HW = H * W      # 6400
NSH = 4         # p-shards per batch
PSH = HW // NSH  # 1600 p rows per shard
PBLK = 13        # p blocks of 128 (1664 = 13*128, last 64 rows are zero-pad)
PPAD = PBLK * 128  # 1664
ROUND = 1024     # q per exp round (2 psum banks)
NROUND = 7       # 6*1024 + 256
FINA_CHUNKS = ((0, 512), (512, 512), (1024, 512), (1536, 64))  # per q-group

_CACHE = {}


def _resize_bilinear_ac(x, h_out, w_out):
    """numpy mirror of the reference's align_corners=True bilinear resize."""
    n, c, h, w = x.shape
    if (h, w) == (h_out, w_out):
        return x
    ys = np.linspace(0.0, h - 1.0, h_out, dtype=np.float32)
    xs = np.linspace(0.0, w - 1.0, w_out, dtype=np.float32)
    y0 = np.floor(ys).astype(np.int32)
    x0 = np.floor(xs).astype(np.int32)
    y1 = np.minimum(y0 + 1, h - 1)
    x1 = np.minimum(x0 + 1, w - 1)
    wy = (ys - y0.astype(np.float32))[None, None, :, None]
    wx = (xs - x0.astype(np.float32))[None, None, None, :]
    g = lambda yi, xi: x[:, :, yi, :][:, :, :, xi]
    top = g(y0, x0) * (1.0 - wx) + g(y0, x1) * wx
    bot = g(y1, x0) * (1.0 - wx) + g(y1, x1) * wx
    return (top * (1.0 - wy) + bot * wy).astype(np.float32)


def _build_bass():
    import concourse.bass as bass
    import concourse.tile as tile
    from concourse import bacc, mybir

    f32 = mybir.dt.float32
    f16 = mybir.dt.float16

    nc = bacc.Bacc(
        "TRN2", target_bir_lowering=False, debug=False, num_devices=N_CORES
    )

    f1p_d = nc.dram_tensor("f1p", [128, PPAD], f16, kind="ExternalInput")
    f2p_d = nc.dram_tensor("f2p", [12, HW], f16, kind="ExternalInput")
    vt_d = nc.dram_tensor("vt", [128, NCLS * PBLK], f32, kind="ExternalInput")
    res_d = nc.dram_tensor("res", [4 * NCLS, PSH], f32, kind="ExternalOutput")

    EXP = mybir.ActivationFunctionType.Exp
    ADD = mybir.AluOpType.add
    MULT = mybir.AluOpType.mult
    AXX = mybir.AxisListType.X

    with tile.TileContext(nc) as tc:
        with (
            tc.tile_pool(name="const", bufs=1) as cpool,
            tc.tile_pool(name="estrip", bufs=2) as epool,
            tc.tile_pool(name="zpool", bufs=2) as zpool,
            tc.tile_pool(name="spsum", bufs=2, space="PSUM") as spool,
            tc.tile_pool(name="fpsum", bufs=1, space="PSUM") as fpool,
        ):
            f1s = cpool.tile([128, PPAD], f16, tag="f1s")
            f2s = cpool.tile([128, HW], f16, tag="f2s")
            vts = cpool.tile([128, NCLS * PBLK], f32, tag="vts")
            outs = cpool.tile([128, PSH], f32, tag="outs")
            # ping-pong [128, 4] lhsT tiles for fina
            vtpA = cpool.tile([128, NCLS], f16, tag="vtpA")
            vtpB = cpool.tile([128, NCLS], f16, tag="vtpB")
            bneg = cpool.tile([128, 1], f32, tag="bneg")
            nc.gpsimd.memset(bneg[:, :], -5.0)

            # inputs, ordered so block 0 round 0 unblocks earliest.  f2's pad
            # rows 12:128 multiply f1's zero rows, so they only need to be
            # finite: memset them on the (idle) Vector/GpSimd engines chunk by
            # chunk instead of DMAing 1.5MB of host zeros.  Rows 0:32 are
            # memset too (32-aligned partition base), then the 12 live rows
            # are DMA'd over them.
            nc.sync.dma_start(out=f1s[:, 0:128], in_=f1p_d[:, 0:128])
            for c in range(NROUND):
                q0 = ROUND * c
                width = min(ROUND, HW - q0)
                eng = nc.vector if c % 2 == 0 else nc.gpsimd
                eng.memset(f2s[:, q0 : q0 + width], 0.0)
                nc.sync.dma_start(
                    out=f2s[0:12, q0 : q0 + width], in_=f2p_d[:, q0 : q0 + width]
                )
            nc.sync.dma_start(out=f1s[:, 128:PPAD], in_=f1p_d[:, 128:PPAD])
            nc.sync.dma_start(out=vts[:, :], in_=vt_d[:, :])

            # persistent fina accumulator: 4 PSUM banks, col-stacked layout
            # (partitions 32g+c, cols 0:1600) = fina[c, 1600g + col]
            finap = fpool.tile([128, 4 * 512], f32, tag="finap")

            ets = [None, None]  # E strips of the last two blocks
            vtps = [vtpA, vtpB]

            def emit_fina(pb, g):
                """fina matmuls for block pb, q-group g, accumulating into
                finap.  One g-group (4 matmuls) at a time so the in-order PE
                queue never blocks the next S round for long."""
                et = ets[pb % 2]
                vtp = vtps[pb % 2]
                for off, w in FINA_CHUNKS:
                    nc.tensor.matmul(
                        finap[32 * g : 32 * g + NCLS, off : off + w],
                        lhsT=vtp[:, :],
                        rhs=et[:, 1600 * g + off : 1600 * g + off + w],
                        start=(pb == 0),
                        stop=(pb == PBLK - 1),
                        tile_position=(0, 32 * g),
                    )

            for pb in range(PBLK):
                et = epool.tile([128, HW], f16, tag="et")
                ets[pb % 2] = et
                zparts = zpool.tile([128, NROUND], f32, tag="zparts")
                rz = zpool.tile([128, 1], f32, tag="rz")
                vtp = vtps[pb % 2]

                for r in range(NROUND):
                    q0 = ROUND * r
                    width = min(ROUND, HW - q0)
                    st = spool.tile([128, ROUND], f32, tag="st")
                    for half in range((width + 511) // 512):
                        qo = q0 + 512 * half
                        qw = min(512, HW - qo)
                        nc.tensor.matmul(
                            st[:, 512 * half : 512 * half + qw],
                            lhsT=f1s[:, 128 * pb : 128 * pb + 128],
                            rhs=f2s[:, qo : qo + qw],
                            start=True,
                            stop=True,
                        )
                    # bias -5: keeps exp within fp16 range (softmax is
                    # shift-invariant; Z accumulates the same shifted values).
                    # Z partials: rounds 0-4 reduced from the E strip on the
                    # Vector engine, rounds 5-6 via the activation accumulator
                    # (the 182ns accumulator read costs Scalar-engine time,
                    # and Scalar is the kernel bottleneck).
                    use_act_accum = r >= 5
                    nc.scalar.activation(
                        et[:, q0 : q0 + width],
                        st[:, 0:width],
                        EXP,
                        bias=bneg[:, 0:1],
                        accum_out=zparts[:, r : r + 1] if use_act_accum else None,
                    )
                    if r < 5:
                        nc.vector.tensor_reduce(
                            zparts[:, r : r + 1], et[:, q0 : q0 + width], AXX, ADD
                        )

                    # software pipeline: previous block's fina matmuls run
                    # on the PE while this block's exps churn, one q-group
                    # per round so the PE injections stay small
                    if 1 <= r <= 4 and pb > 0:
                        emit_fina(pb - 1, r - 1)

                # Z = sum of round partials; vtp = vt[:, block] / Z * 2048
                nc.vector.tensor_reduce(rz[:, 0:1], zparts[:, :], AXX, ADD)
                nc.vector.reciprocal(rz[:, 0:1], rz[:, 0:1])
                nc.vector.tensor_scalar(
                    vtp[:, 0:NCLS],
                    vts[:, NCLS * pb : NCLS * pb + NCLS],
                    rz[:, 0:1],
                    2048.0,
                    MULT,
                    MULT,
                )

            # last block's fina, chunk-major so each column range finishes
            # across all 4 q-groups and can be drained while later chunks
            # still run on the PE (PSUM -> SBUF via Vector; DMA can't read
            # PSUM)
            et = ets[(PBLK - 1) % 2]
            vtp = vtps[(PBLK - 1) % 2]
            for off, w in FINA_CHUNKS:
                for g in range(4):
                    nc.tensor.matmul(
                        finap[32 * g : 32 * g + NCLS, off : off + w],
                        lhsT=vtp[:, :],
                        rhs=et[:, 1600 * g + off : 1600 * g + off + w],
                        start=False,
                        stop=True,
                        tile_position=(0, 32 * g),
                    )
                nc.vector.tensor_copy(
                    outs[:, off : off + w], finap[:, off : off + w]
                )
                for g in range(4):
                    nc.sync.dma_start(
                        out=res_d[NCLS * g : NCLS * g + NCLS, off : off + w],
                        in_=outs[32 * g : 32 * g + NCLS, off : off + w],
                    )

    nc.compile()
    return nc


def _get_nc():
    if "nc" not in _CACHE:
        _CACHE["nc"] = _build_bass()
    return _CACHE["nc"]


def _hilo16(x):
    """fp16 high/low split: x ~= hi + lo exactly to ~2^-22 relative."""
    x = np.asarray(x, np.float32)
    hi = x.astype(np.float16)
    lo = (x - hi.astype(np.float32)).astype(np.float16)
    return hi, lo


def _prep_inputs(feature_in, out, w1, b1, w2, b2):
    feature_in = np.asarray(feature_in, np.float32)
    out = np.asarray(out, np.float32)
    w1 = np.asarray(w1, np.float32)
    b1 = np.asarray(b1, np.float32)
    w2 = np.asarray(w2, np.float32)
    b2 = np.asarray(b2, np.float32)

    scale = np.float32(1.0 / np.sqrt(NCLS))
    feat = feature_in.reshape(NB, C_IN, HW)
    # f1 carries the softmax scale; f2 is plain
    f1 = (np.einsum("oc,ncp->nop", w1, feat, dtype=np.float32) + b1[None, :, None]) * scale
    f2 = np.einsum("oc,ncp->nop", w2, feat, dtype=np.float32) + b2[None, :, None]
    f1 = f1.astype(np.float32)
    f2 = f2.astype(np.float32)
    v = _resize_bilinear_ac(out, H, W).reshape(NB, NCLS, HW)

    in_maps = []
    for core in range(N_CORES):
        b, s = divmod(core, NSH)
        p0 = PSH * s
        f1p = np.zeros((128, PPAD), np.float16)
        h1, l1 = _hilo16(f1[b][:, p0 : p0 + PSH])
        f1p[0:4, :PSH] = h1
        f1p[4:8, :PSH] = l1
        f1p[8:12, :PSH] = h1
        h2, l2 = _hilo16(f2[b])
        f2p = np.concatenate([h2, h2, l2], axis=0)  # [12, HW] fp16
        vtp = np.zeros((NCLS, PPAD), np.float32)
        vtp[:, :PSH] = v[b][:, p0 : p0 + PSH]
        # vt[part, 4*pb + c] = V[c, p0 + 128*pb + part]
        vt = vtp.reshape(NCLS, PBLK, 128).transpose(2, 1, 0).reshape(128, PBLK * NCLS)
        in_maps.append(
            {
                "f1p": f1p,
                "f2p": np.ascontiguousarray(f2p),
                "vt": np.ascontiguousarray(vt),
            }
        )
    return in_maps


def _unpack(results):
    """results: list of 8 dicts with 'res' [16, 1600] -> fina [2,4,80,80]."""
    fina = np.zeros((NB, NCLS, HW), np.float32)
    for core in range(N_CORES):
        b, s = divmod(core, NSH)
        res = np.asarray(results[core]["res"], np.float32)  # [16, 1600]
        part = res.reshape(4, NCLS, PSH)  # [q-group g, class j, cols]
        for g in range(4):
            fina[b, :, PSH * g : PSH * g + PSH] += part[g]
    fina *= np.float32(1.0 / 2048.0)
    return fina.reshape(NB, NCLS, H, W)


def run(inputs, trace=False):
    from concourse.bass_utils import run_bass_kernel_spmd

    nc = _get_nc()
    in_maps = _prep_inputs(**inputs)
    r = run_bass_kernel_spmd(nc, in_maps, list(range(N_CORES)), trace=trace)
    return _unpack(r.results), r.exec_time_ns


def kernel(feature_in, out, w1, b1, w2, b2):
    result, _ = run(
        dict(feature_in=feature_in, out=out, w1=w1, b1=b1, w2=w2, b2=b2)
    )
    return result


# revision 29
# speedup vs baseline: 1.0443x; 1.0443x over previous
"""Trainium2 Bass kernel for nn_Corr (correlation-attention module).

Math (per batch n):
    f1 = 0.5*(w1 @ feat + b1)        # [4, 6400]   feat = feature_in[n] flattened
    f2 =      w2 @ feat + b2         # [4, 6400]
    S  = f1^T @ f2                   # [6400, 6400]  (0.5 = 1/sqrt(nclass) folded into f1)
    A  = softmax(S, axis=1)          # row softmax (over q)
    V  = bilinear_resize(out[n])     # [4, 6400]
    fina[c, q] = sum_p V[c, p]/Z_p * exp(S[p, q])

Sharding: 2 batches x 4 p-shards (rows of S) = 8 cores. Each core produces a
partial fina over its 1600 p-rows; host sums the 4 partials per batch.

Device kernel per core (p-shard of 1664 rows incl pad, all 6400 q):
  - S chunk = matmul(lhsT=f1[12, pblock-128cols], rhs=f2[12, qchunk]) with
    K=12 (hi/lo fp16 split rows, no zero padding needed) and M=128: one rhs
    stream per 512 cols.  PSUM: 2 ping-pong [128,1024] round buffers.
  - exp on ScalarE PSUM->SBUF (1024-wide) with accum_out giving row sums Z.
  - fina = matmul(lhsT=vt*recip(Z) [128,4], rhs=E chunks) accumulated across
    all 13 p-blocks directly in a persistent 4-bank PSUM tile (start on
    pb==0, stop on pb==12), col-stacked: partition group 32g holds q-group g.
  - fina matmuls of block b-1 are emitted inside block b's round loop
    (software pipeline) so the Scalar engine never starves.
"""

import numpy as np

N_CORES = 8
NB = 2          # batches
NCLS = 4        # nclass
C_IN = 32
H = W = 80
HW = H * W      # 6400
NSH = 4         # p-shards per batch
PSH = HW // NSH  # 1600 p rows per shard
PBLK = 13        # p blocks of 128 (1664 = 13*128, last 64 rows are zero-pad)
PPAD = PBLK * 128  # 1664
ROUND = 1024     # q per exp round (2 psum banks)
NROUND = 7       # 6*1024 + 256
FINA_CHUNKS = ((0, 512), (512, 512), (1024, 512), (1536, 64))  # per q-group

_CACHE = {}


def _resize_bilinear_ac(x, h_out, w_out):
    """numpy mirror of the reference's align_corners=True bilinear resize."""
    n, c, h, w = x.shape
    if (h, w) == (h_out, w_out):
        return x
    ys = np.linspace(0.0, h - 1.0, h_out, dtype=np.float32)
    xs = np.linspace(0.0, w - 1.0, w_out, dtype=np.float32)
    y0 = np.floor(ys).astype(np.int32)
    x0 = np.floor(xs).astype(np.int32)
    y1 = np.minimum(y0 + 1, h - 1)
    x1 = np.minimum(x0 + 1, w - 1)
    wy = (ys - y0.astype(np.float32))[None, None, :, None]
    wx = (xs - x0.astype(np.float32))[None, None, None, :]
    g = lambda yi, xi: x[:, :, yi, :][:, :, :, xi]
    top = g(y0, x0) * (1.0 - wx) + g(y0, x1) * wx
    bot = g(y1, x0) * (1.0 - wx) + g(y1, x1) * wx
    return (top * (1.0 - wy) + bot * wy).astype(np.float32)


def _build_bass():
    import concourse.bass as bass
    import concourse.tile as tile
    from concourse import bacc, mybir

    f32 = mybir.dt.float32
    f16 = mybir.dt.float16

    nc = bacc.Bacc(
        "TRN2", target_bir_lowering=False, debug=False, num_devices=N_CORES
    )

    f1p_d = nc.dram_tensor("f1p", [128, PPAD], f16, kind="ExternalInput")
    f2p_d = nc.dram_tensor("f2p", [12, HW], f16, kind="ExternalInput")
    vt_d = nc.dram_tensor("vt", [128, NCLS * PBLK], f32, kind="ExternalInput")
    # full 128 partitions: only rows 32g+c (c<4) carry data, but a dense
    # full-partition DMA runs ~20x faster than 16 thin 4-partition DMAs
    res_d = nc.dram_tensor("res", [128, PSH], f32, kind="ExternalOutput")

    EXP = mybir.ActivationFunctionType.Exp
    ADD = mybir.AluOpType.add
    MULT = mybir.AluOpType.mult
    AXX = mybir.AxisListType.X

    with tile.TileContext(nc) as tc:
        with (
            tc.tile_pool(name="const", bufs=1) as cpool,
            tc.tile_pool(name="estrip", bufs=2) as epool,
            tc.tile_pool(name="zpool", bufs=2) as zpool,
            tc.tile_pool(name="spsum", bufs=2, space="PSUM") as spool,
            tc.tile_pool(name="fpsum", bufs=1, space="PSUM") as fpool,
        ):
            f1s = cpool.tile([128, PPAD], f16, tag="f1s")
            f2s = cpool.tile([128, HW], f16, tag="f2s")
            vts = cpool.tile([128, NCLS * PBLK], f32, tag="vts")
            outs = cpool.tile([128, PSH], f32, tag="outs")
            # ping-pong [128, 4] lhsT tiles for fina
            vtpA = cpool.tile([128, NCLS], f16, tag="vtpA")
            vtpB = cpool.tile([128, NCLS], f16, tag="vtpB")
            bneg = cpool.tile([128, 1], f32, tag="bneg")
            nc.gpsimd.memset(bneg[:, :], -5.0)

            # inputs, ordered so block 0 round 0 unblocks earliest.  f2's pad
            # rows 12:128 multiply f1's zero rows, so they only need to be
            # finite: memset them on the (idle) Vector/GpSimd engines chunk by
            # chunk instead of DMAing 1.5MB of host zeros.  Rows 0:32 are
            # memset too (32-aligned partition base), then the 12 live rows
            # are DMA'd over them.
            nc.sync.dma_start(out=f1s[:, 0:128], in_=f1p_d[:, 0:128])
            for c in range(NROUND):
                q0 = ROUND * c
                width = min(ROUND, HW - q0)
                eng = nc.gpsimd if c % 2 == 0 else nc.vector
                eng.memset(f2s[:, q0 : q0 + width], 0.0)
                nc.sync.dma_start(
                    out=f2s[0:12, q0 : q0 + width], in_=f2p_d[:, q0 : q0 + width]
                )
            nc.sync.dma_start(out=f1s[:, 128:PPAD], in_=f1p_d[:, 128:PPAD])
            nc.sync.dma_start(out=vts[:, :], in_=vt_d[:, :])

            # persistent fina accumulator: 4 PSUM banks, col-stacked layout
            # (partitions 32g+c, cols 0:1600) = fina[c, 1600g + col]
            finap = fpool.tile([128, 4 * 512], f32, tag="finap")

            ets = [None, None]  # E strips of the last two blocks
            vtps = [vtpA, vtpB]

            def emit_fina(pb, g):
                """fina matmuls for block pb, q-group g, accumulating into
                finap.  One g-group (4 matmuls) at a time so the in-order PE
                queue never blocks the next S round for long."""
                et = ets[pb % 2]
                vtp = vtps[pb % 2]
                for off, w in FINA_CHUNKS:
                    nc.tensor.matmul(
                        finap[32 * g : 32 * g + NCLS, off : off + w],
                        lhsT=vtp[:, :],
                        rhs=et[:, 1600 * g + off : 1600 * g + off + w],
                        start=(pb == 0),
                        stop=(pb == PBLK - 1),
                        tile_position=(0, 32 * g),
                    )

            for pb in range(PBLK):
                et = epool.tile([128, HW], f16, tag="et")
                ets[pb % 2] = et
                zparts = zpool.tile([128, NROUND], f32, tag="zparts")
                rz = zpool.tile([128, 1], f32, tag="rz")
                vtp = vtps[pb % 2]

                for r in range(NROUND):
                    q0 = ROUND * r
                    width = min(ROUND, HW - q0)
                    st = spool.tile([128, ROUND], f32, tag="st")
                    for half in range((width + 511) // 512):
                        qo = q0 + 512 * half
                        qw = min(512, HW - qo)
                        nc.tensor.matmul(
                            st[:, 512 * half : 512 * half + qw],
                            lhsT=f1s[:, 128 * pb : 128 * pb + 128],
                            rhs=f2s[:, qo : qo + qw],
                            start=True,
                            stop=True,
                        )
                    # bias -5: keeps exp within fp16 range (softmax is
                    # shift-invariant; Z accumulates the same shifted values).
                    # Z partials: rounds 0-4 reduced from the E strip on the
                    # Vector engine, rounds 5-6 via the activation accumulator
                    # (the 182ns accumulator read costs Scalar-engine time,
                    # and Scalar is the kernel bottleneck).
                    use_act_accum = r >= 5
                    nc.scalar.activation(
                        et[:, q0 : q0 + width],
                        st[:, 0:width],
                        EXP,
                        bias=bneg[:, 0:1],
                        accum_out=zparts[:, r : r + 1] if use_act_accum else None,
                    )
                    if r < 5:
                        nc.vector.tensor_reduce(
                            zparts[:, r : r + 1], et[:, q0 : q0 + width], AXX, ADD
                        )

                    # software pipeline: previous block's fina matmuls run
                    # on the PE while this block's exps churn, one q-group
                    # per round so the PE injections stay small
                    if 1 <= r <= 4 and pb > 0:
                        emit_fina(pb - 1, r - 1)

                # Z = sum of round partials; vtp = vt[:, block] / Z * 2048
                nc.vector.tensor_reduce(rz[:, 0:1], zparts[:, :], AXX, ADD)
                nc.vector.reciprocal(rz[:, 0:1], rz[:, 0:1])
                nc.vector.tensor_scalar(
                    vtp[:, 0:NCLS],
                    vts[:, NCLS * pb : NCLS * pb + NCLS],
                    rz[:, 0:1],
                    2048.0,
                    MULT,
                    MULT,
                )

            # last block's fina, chunk-major so each column range finishes
            # across all 4 q-groups and can be drained while later chunks
            # still run on the PE (PSUM -> SBUF via Vector; DMA can't read
            # PSUM)
            et = ets[(PBLK - 1) % 2]
            vtp = vtps[(PBLK - 1) % 2]
            for off, w in FINA_CHUNKS:
                for g in range(4):
                    nc.tensor.matmul(
                        finap[32 * g : 32 * g + NCLS, off : off + w],
                        lhsT=vtp[:, :],
                        rhs=et[:, 1600 * g + off : 1600 * g + off + w],
                        start=False,
                        stop=True,
                        tile_position=(0, 32 * g),
                    )
                nc.vector.tensor_copy(
                    outs[:, off : off + w], finap[:, off : off + w]
                )
                nc.sync.dma_start(
                    out=res_d[:, off : off + w],
                    in_=outs[:, off : off + w],
                )

    nc.compile()
    return nc


def _get_nc():
    if "nc" not in _CACHE:
        _CACHE["nc"] = _build_bass()
    return _CACHE["nc"]


def _hilo16(x):
    """fp16 high/low split: x ~= hi + lo exactly to ~2^-22 relative."""
    x = np.asarray(x, np.float32)
    hi = x.astype(np.float16)
    lo = (x - hi.astype(np.float32)).astype(np.float16)
    return hi, lo


def _prep_inputs(feature_in, out, w1, b1, w2, b2):
    feature_in = np.asarray(feature_in, np.float32)
    out = np.asarray(out, np.float32)
    w1 = np.asarray(w1, np.float32)
    b1 = np.asarray(b1, np.float32)
    w2 = np.asarray(w2, np.float32)
    b2 = np.asarray(b2, np.float32)

    scale = np.float32(1.0 / np.sqrt(NCLS))
    feat = feature_in.reshape(NB, C_IN, HW)
    # f1 carries the softmax scale; f2 is plain
    f1 = (np.einsum("oc,ncp->nop", w1, feat, dtype=np.float32) + b1[None, :, None]) * scale
    f2 = np.einsum("oc,ncp->nop", w2, feat, dtype=np.float32) + b2[None, :, None]
    f1 = f1.astype(np.float32)
    f2 = f2.astype(np.float32)
    v = _resize_bilinear_ac(out, H, W).reshape(NB, NCLS, HW)

    in_maps = []
    for core in range(N_CORES):
        b, s = divmod(core, NSH)
        p0 = PSH * s
        f1p = np.zeros((128, PPAD), np.float16)
        h1, l1 = _hilo16(f1[b][:, p0 : p0 + PSH])
        f1p[0:4, :PSH] = h1
        f1p[4:8, :PSH] = l1
        f1p[8:12, :PSH] = h1
        h2, l2 = _hilo16(f2[b])
        f2p = np.concatenate([h2, h2, l2], axis=0)  # [12, HW] fp16
        vtp = np.zeros((NCLS, PPAD), np.float32)
        vtp[:, :PSH] = v[b][:, p0 : p0 + PSH]
        # vt[part, 4*pb + c] = V[c, p0 + 128*pb + part]
        vt = vtp.reshape(NCLS, PBLK, 128).transpose(2, 1, 0).reshape(128, PBLK * NCLS)
        in_maps.append(
            {
                "f1p": f1p,
                "f2p": np.ascontiguousarray(f2p),
                "vt": np.ascontiguousarray(vt),
            }
        )
    return in_maps


def _unpack(results):
    """results: list of 8 dicts with 'res' [16, 1600] -> fina [2,4,80,80]."""
    fina = np.zeros((NB, NCLS, HW), np.float32)
    for core in range(N_CORES):
        b, s = divmod(core, NSH)
        res = np.asarray(results[core]["res"], np.float32)  # [128, 1600]
        for g in range(4):
            fina[b, :, PSH * g : PSH * g + PSH] += res[32 * g : 32 * g + NCLS]
    fina *= np.float32(1.0 / 2048.0)
    return fina.reshape(NB, NCLS, H, W)


def run(inputs, trace=False):
    from concourse.bass_utils import run_bass_kernel_spmd

    nc = _get_nc()
    in_maps = _prep_inputs(**inputs)
    r = run_bass_kernel_spmd(nc, in_maps, list(range(N_CORES)), trace=trace)
    return _unpack(r.results), r.exec_time_ns


def kernel(feature_in, out, w1, b1, w2, b2):
    result, _ = run(
        dict(feature_in=feature_in, out=out, w1=w1, b1=b1, w2=w2, b2=b2)
    )
    return result
